# revision 20
# baseline (speedup 1.0000x reference)
"""BertEmbeddings (word lookup + header mean-pool scatter + pos/type/match
embeddings + TF-style LayerNorm) as a Bass/Tile kernel on 8 trn2 NeuronCores.

Sharding: data-parallel over batch (4 rows/core); embedding tables replicated.

Per-core device pipeline (v8):
  - bf16 word table; word gathers (SWDGE queue 0) and compacted pair-packed
    header gathers (SWDGE queue 1) run with interleaved descriptor
    generation; target pos rows via indirect DMA
  - per 128-token block: pos + small-table adds via bf16 matmuls (multihot +
    eye) into PSUM; emb(bf16) = words + psum via scalar_tensor_tensor whose
    accum_out yields the LN row-sum for free; sum-of-squares on ACT for the
    first half of the blocks and on DVE (bf16 STT) for the second half;
    stats batched per batch row; LN apply on ACT
  - target block: compacted header rows pair-added on DVE (host packs pairs
    of same-slot tokens), then segment-mean-reduced by PE matmuls whose
    one-hot matrix carries 1/len, accumulated with the targets' small-table
    multihot in PSUM; pos rows for targets from an indirect gather;
    bn_stats LayerNorm; bounds-checked indirect scatter-store overwrites the
    affected tokens (invalid slots OOB-skipped)

All data-dependent arithmetic on embedding VALUES runs on device; the host
only reformats index tensors and precomputes input-independent table
derivatives (zero-row padding, one-hot/multi-hot encodings, 1/len weights).
"""

import numpy as np

B, S, H = 32, 512, 768
VOCAB = 30522
NCORES = 8
BPC = B // NCORES            # batch rows per core
T = BPC * S                  # tokens per core
C, L = 32, 16                # columns, max header len
NSLOT = BPC * C              # 128 slots per core
ZROW = VOCAB                 # zero row in augmented word table
WROWS = VOCAB + 1
NV = 19                      # 2 + 11 + 6 small-table rows
EPS = 1e-12

# cA column layout: [eye(128) | posmat(4*768) | Mp(Gp*128)]
CA_EYE = 0
CA_POS = 128
CA_M = 128 + 4 * H
# cB column layout: [mh(T) | small(768) | mht(128)]
CB_MH = 0
CB_SM = T
CB_MHT = T + H
CB_W = CB_MHT + 128

_NC_CACHE = {}


def _build_nc(skip_affine: bool, Gp: int):
    from contextlib import ExitStack

    import concourse.bacc as bacc
    import concourse.tile as tile
    from concourse import mybir

    BF16 = mybir.dt.bfloat16
    I16 = mybir.dt.int16
    I32 = mybir.dt.int32
    F32 = mybir.dt.float32

    nc = bacc.Bacc(
        "TRN2", target_bir_lowering=False, debug=False, num_swdge_queues=4,
        dynamic_dma_scratch_size=32768,
    )
    t = {}

    def inp(name, shape, dt=F32):
        t[name] = nc.dram_tensor(name, shape, dt, kind="ExternalInput").ap()

    inp("word_aug", [WROWS, H], BF16)
    inp("pos_aug", [S, H], BF16)
    inp("idx16", [128, 128 + Gp * 16], I16)  # [widx16 | hidx pairs], wrap16
    inp("idx32", [128, 2], I32)              # [posidx | tgtrow]
    inp("cA", [128, CA_M + Gp * 128], BF16)  # eye | pos rows | seg matrix
    inp("cB", [NV, CB_W], BF16)              # multihot | small | tgt multihot
    if not skip_affine:
        inp("lnw", [1, H])
        inp("lnb", [1, H])
    out = nc.dram_tensor("out", [BPC, S, H], F32, kind="ExternalOutput").ap()

    with tile.TileContext(nc) as tc, ExitStack() as ctx:
        _body(ctx, tc, t, out, skip_affine, Gp, mybir)
    nc.compile()
    return nc


def _body(ctx, tc, t, out, skip_affine, Gp, mybir):
    import concourse.bass as bass
    from concourse.tile import add_dep_helper

    nc = tc.nc
    F32 = mybir.dt.float32
    BF16 = mybir.dt.bfloat16
    MUL = mybir.AluOpType.mult
    ADD = mybir.AluOpType.add
    SUB = mybir.AluOpType.subtract
    AF = mybir.ActivationFunctionType

    const = ctx.enter_context(tc.tile_pool(name="const", bufs=1))
    setup = ctx.enter_context(tc.tile_pool(name="setup", bufs=1))
    hpool = ctx.enter_context(tc.tile_pool(name="hdr", bufs=1))
    wpool = ctx.enter_context(tc.tile_pool(name="wrd", bufs=4))
    epool = ctx.enter_context(tc.tile_pool(name="emb", bufs=16))
    opool = ctx.enter_context(tc.tile_pool(name="outp", bufs=16))
    qpool = ctx.enter_context(tc.tile_pool(name="sq", bufs=2))
    spool = ctx.enter_context(tc.tile_pool(name="stat", bufs=4))
    psum = ctx.enter_context(tc.tile_pool(name="ps", bufs=4, space="PSUM"))

    # ---- index tiles first; alone on the SP HWDGE ring so they land fast
    s_idx16 = const.tile([128, 128 + Gp * 16], mybir.dt.int16)
    nc.sync.dma_start(s_idx16[:], t["idx16"])
    s_idx32 = const.tile([128, 2], mybir.dt.int32)
    nc.sync.dma_start(s_idx32[:], t["idx32"])

    # ---- word gathers: ch1-3 async on queues 1-3, ch0 on the mainline ----
    wchs = [None] * BPC
    word_gathers = [None] * BPC
    for ch in (1, 2, 3, 0):
        wch = wpool.tile([128, 4, H], BF16)
        g = nc.gpsimd.dma_gather(
            wch[:], t["word_aug"], s_idx16[:, 32 * ch : 32 * (ch + 1)],
            512, 512, H, queue_num=ch,
        )
        word_gathers[ch] = g
        wchs[ch] = wch

    # ----- compacted pair-packed header gathers (SWDGE queue 1) ----------
    GH1 = (2 * Gp + 1) // 2          # chunks in first header gather
    GH2 = 2 * Gp - GH1
    hch1 = hpool.tile([128, GH1, H], BF16, tag="h1")
    hg = nc.gpsimd.dma_gather(
        hch1[:], t["word_aug"], s_idx16[:, 128 : 128 + GH1 * 8],
        GH1 * 128, GH1 * 128, H, queue_num=1,
    )
    add_dep_helper(hg.ins, word_gathers[1].ins, sync=False,
                   reason="SWDGE q1 order")
    hdr_gathers = [hg]
    if GH2 > 0:
        hch2 = hpool.tile([128, GH2, H], BF16, tag="h2")
        g = nc.gpsimd.dma_gather(
            hch2[:], t["word_aug"],
            s_idx16[:, 128 + GH1 * 8 : 128 + 2 * Gp * 8],
            GH2 * 128, GH2 * 128, H, queue_num=2,
        )
        add_dep_helper(g.ins, word_gathers[2].ins, sync=False,
                       reason="SWDGE q2 order")
        hdr_gathers.append(g)

    def hcol(c):
        if c < GH1:
            return hch1[:, c, :]
        return hch2[:, c - GH1, :]

    # target tokens' pos rows
    postgt = setup.tile([128, H], BF16)
    g = nc.gpsimd.indirect_dma_start(
        postgt[:], None, t["pos_aug"],
        bass.IndirectOffsetOnAxis(ap=s_idx32[:, 0:1], axis=0),
    )
    add_dep_helper(g.ins, word_gathers[-1].ins, sync=False,
                   reason="after word gathers on q0")

    # ---- big consts on the ACT HWDGE ring (don't delay idx on SP) -------
    s_cA = const.tile([128, CA_M + Gp * 128], BF16)
    nc.scalar.dma_start(s_cA[:], t["cA"])
    s_cB = const.tile([NV, CB_W], BF16)
    nc.scalar.dma_start(s_cB[:], t["cB"])

    s_eps = const.tile([128, 1], F32)
    nc.vector.memset(s_eps[:], EPS)

    if not skip_affine:
        s_lnw = const.tile([128, H], F32)
        gg = nc.gpsimd.dma_start(s_lnw[:], t["lnw"].partition_broadcast(128))
        add_dep_helper(gg.ins, g.ins, sync=False,
                       reason="gathers first on SWDGE queue")
        s_lnb = const.tile([128, H], F32)
        nc.gpsimd.dma_start(s_lnb[:], t["lnb"].partition_broadcast(128))

    s_eye = s_cA[:, CA_EYE:CA_EYE + 128]
    s_small = s_cB[:, CB_SM:CB_SM + H]

    # ---------------- main token blocks (no header dependency) -----------
    inv_h = 1.0 / H
    stores = []
    hps = []
    last_mm = None
    last_dve = None
    last_act = None
    for ch in range(BPC):
        wch = wchs[ch]
        usum = spool.tile([128, 4], F32, tag="usum")
        rsq = spool.tile([128, 4], F32, tag="rsq")
        embs = []
        for jj in range(4):
            j = ch * 4 + jj
            ps = psum.tile([128, H], F32, tag="ps")
            lhs_mh = s_cB[:, CB_MH + j * 128 : CB_MH + (j + 1) * 128]
            pos_on_pe = ch >= 2
            for lo, hi in ((0, 512), (512, H)):
                nc.tensor.matmul(
                    ps[:, lo:hi], lhs_mh, s_small[:, lo:hi],
                    start=True, stop=not pos_on_pe,
                )
                if pos_on_pe:
                    last_mm = nc.tensor.matmul(
                        ps[:, lo:hi], s_eye,
                        s_cA[:, CA_POS + jj * H + lo : CA_POS + jj * H + hi],
                        start=False, stop=True,
                    )

            # emb = words (+ pos) + ps; accum_out feeds the LN mean
            emb = epool.tile([128, H], BF16)
            if pos_on_pe:
                wsrc = wch[:, jj, :]
            else:
                wsrc = epool.tile([128, H], BF16, tag="wp")
                nc.vector.tensor_add(
                    wsrc[:], wch[:, jj, :],
                    s_cA[:, CA_POS + jj * H : CA_POS + (jj + 1) * H],
                )
                wsrc = wsrc[:]
            last_dve = nc.vector.scalar_tensor_tensor(
                emb[:], wsrc if pos_on_pe else wsrc, 1.0, ps[:, 0:H],
                op0=MUL, op1=ADD, accum_out=usum[:, jj : jj + 1],
            )
            embs.append(emb)

            sq = qpool.tile([128, H], BF16)
            last_act = nc.scalar.activation(
                sq[:], emb[:], AF.Square, accum_out=rsq[:, jj : jj + 1]
            )

        # batched LN stats for the 4 blocks of this batch row
        uneg = spool.tile([128, 4], F32, tag="uneg")
        nc.vector.tensor_scalar_mul(uneg[:], usum[:], -inv_h)
        sq2 = spool.tile([128, 4], F32, tag="sq2")
        nc.vector.tensor_mul(sq2[:], uneg[:], uneg[:])
        var = spool.tile([128, 4], F32, tag="var")
        nc.vector.scalar_tensor_tensor(
            var[:], rsq[:], inv_h, sq2[:], op0=MUL, op1=SUB
        )
        rstd = spool.tile([128, 4], F32, tag="rstd")
        last_act = nc.scalar.activation(
            rstd[:], var[:], AF.Sqrt, bias=s_eps[:], scale=1.0
        )
        nc.vector.reciprocal(rstd[:], rstd[:])
        nub = spool.tile([128, 4], F32, tag="nub")
        if ch == 0:
            last_dve = nc.vector.tensor_mul(nub[:], uneg[:], rstd[:])
        else:
            nc.gpsimd.tensor_mul(nub[:], uneg[:], rstd[:])

        for jj in range(4):
            o = opool.tile([128, H], F32)
            if ch == 0 or (ch == 3 and jj % 2 == 1):
                last_act = nc.scalar.activation(
                    o[:], embs[jj][:], AF.Identity,
                    bias=nub[:, jj : jj + 1], scale=rstd[:, jj : jj + 1],
                )
            else:
                nc.gpsimd.tensor_scalar(
                    o[:], embs[jj][:], rstd[:, jj : jj + 1],
                    nub[:, jj : jj + 1], op0=MUL, op1=ADD,
                )
            if not skip_affine:
                nc.vector.tensor_mul(o[:], o[:], s_lnw[:])
                nc.vector.tensor_add(o[:], o[:], s_lnb[:])
            stores.append(
                nc.sync.dma_start(out[ch, jj * 128 : (jj + 1) * 128, :], o[:])
            )

    # -------- target block: pooled headers + pos + small -----------------
    # pair-add same-slot header tokens (host packed them adjacently)
    for gp in range(Gp):
        hp = setup.tile([128, H], BF16, tag=f"hp{gp}")
        nc.vector.tensor_add(hp[:], hcol(2 * gp), hcol(2 * gp + 1))
        hps.append(hp)

    ps_t = psum.tile([128, H], F32, tag="ps")
    for lo, hi in ((0, 512), (512, H)):
        mm = nc.tensor.matmul(
            ps_t[:, lo:hi], s_cB[:, CB_MHT:CB_MHT + 128], s_small[:, lo:hi],
            start=True, stop=False,
        )
        if lo == 0:
            add_dep_helper(mm.ins, last_mm.ins, sync=False,
                           reason="target matmuls after main matmuls")
        # segment mean-pool of the pair-added header rows (1/len in cA)
        for gp in range(Gp):
            nc.tensor.matmul(
                ps_t[:, lo:hi],
                s_cA[:, CA_M + gp * 128 : CA_M + (gp + 1) * 128],
                hps[gp][:, lo:hi],
                start=False, stop=(gp == Gp - 1),
            )

    emb_t = setup.tile([128, H], F32)
    nc.vector.scalar_tensor_tensor(
        emb_t[:], postgt[:], 1.0, ps_t[:, 0:H], op0=MUL, op1=ADD
    )

    stats = setup.tile([128, 2, 6], F32)
    nc.vector.bn_stats(stats[:, 0, :], emb_t[:, 0:384])
    nc.vector.bn_stats(stats[:, 1, :], emb_t[:, 384:768])
    mv = setup.tile([128, 2], F32)
    nc.vector.bn_aggr(mv[:], stats[:])
    rstd_t = setup.tile([128, 1], F32)
    sa = nc.scalar.activation(
        rstd_t[:], mv[:, 1:2], AF.Sqrt, bias=s_eps[:], scale=1.0
    )
    add_dep_helper(sa.ins, last_act.ins, sync=False,
                   reason="target ACT after main ACT")
    nc.vector.reciprocal(rstd_t[:], rstd_t[:])
    o_t = setup.tile([128, H], F32)
    nc.vector.tensor_scalar(
        o_t[:], emb_t[:], mv[:, 0:1], rstd_t[:], op0=SUB, op1=MUL
    )
    if not skip_affine:
        nc.vector.tensor_mul(o_t[:], o_t[:], s_lnw[:])
        nc.vector.tensor_add(o_t[:], o_t[:], s_lnb[:])

    # overwrite the scattered rows; invalid slots point out of bounds and
    # are silently skipped via the bounds check
    scat = nc.gpsimd.indirect_dma_start(
        out.rearrange("b s h -> (b s) h"),
        bass.IndirectOffsetOnAxis(ap=s_idx32[:, 1:2], axis=0),
        o_t[:], None,
        bounds_check=T - 1, oob_is_err=False,
    )
    for st in stores:
        add_dep_helper(
            scat.ins, st.ins, sync=True, reason="scatter after block stores"
        )


def _wrap16(flat):
    w = flat.reshape(-1, 16).T.astype(np.int16)
    return np.tile(w, (8, 1))


def _multihot(tt, mt, ti, n, dtype):
    mh1 = np.zeros((NV, n), dtype=dtype)
    ar = np.arange(n)
    mh1[tt, ar] = 1
    mh1[2 + mt, ar] += 1
    mh1[13 + ti, ar] += 1
    return mh1


def _prep_core(core, iid, hdr, tt, mt, ti, cpos, cidx, hlen, bf16):
    b0 = core * BPC
    sl = slice(b0, b0 + BPC)
    iids = iid[sl]

    widx16 = _wrap16(iids.reshape(-1))

    bb = np.arange(BPC)[:, None]
    sel_hdr = hdr[sl][bb, cidx[sl]]                      # [BPC, C, L]
    sel_len = hlen[sl][bb, cidx[sl]]                     # [BPC, C]

    # pair-pack valid header tokens: each pair holds 2 tokens of one slot
    pairs = []                                           # (slot, tok0, tok1)
    tok = sel_hdr.reshape(NSLOT, L)
    lens = sel_len.reshape(NSLOT)
    for s in range(NSLOT):
        ln = int(lens[s])
        for i in range(0, ln, 2):
            t0 = int(tok[s, i])
            t1 = int(tok[s, i + 1]) if i + 1 < ln else ZROW
            pairs.append((s, t0, t1))

    valid = lens > 0
    posidx = cpos[sl].reshape(NSLOT).astype(np.int32)
    tgtrow = np.where(
        valid, (bb * S + cpos[sl]).reshape(-1), 10 * T
    ).astype(np.int32)

    ttf, mtf, tif = tt[sl].reshape(-1), mt[sl].reshape(-1), ti[sl].reshape(-1)
    mh = _multihot(ttf, mtf, tif, T, bf16)

    tt_t = tt[sl][bb, cpos[sl]].reshape(-1)
    mt_t = mt[sl][bb, cpos[sl]].reshape(-1)
    ti_t = ti[sl][bb, cpos[sl]].reshape(-1)
    mht = _multihot(tt_t, mt_t, ti_t, NSLOT, bf16)

    return widx16, pairs, lens, posidx, tgtrow, mh, mht


def make_in_maps(inputs):
    import ml_dtypes

    bf16 = ml_dtypes.bfloat16
    inp = {k: np.asarray(v) for k, v in inputs.items()}
    word = np.ascontiguousarray(inp["word_emb"], dtype=np.float32)
    word_aug = np.concatenate(
        [word.astype(bf16), np.zeros((1, H), bf16)], axis=0
    )

    small16 = np.concatenate(
        [inp["tok_type_emb"], inp["match_emb"], inp["type_emb"]], axis=0
    ).astype(np.float32).astype(bf16)                    # [19, H]

    pos16 = np.ascontiguousarray(inp["pos_emb"], dtype=np.float32).astype(bf16)
    posmat = pos16.reshape(4, 128, H).transpose(1, 0, 2).reshape(128, 4 * H)
    eye = np.eye(128, dtype=bf16)

    lnw = np.ascontiguousarray(inp["ln_w"], dtype=np.float32).reshape(1, H)
    lnb = np.ascontiguousarray(inp["ln_b"], dtype=np.float32).reshape(1, H)
    skip_affine = bool(np.all(lnw == 1.0) and np.all(lnb == 0.0))

    iid = inp["input_ids"].astype(np.int64)
    hdr = inp["header_ids"].astype(np.int64)
    tt = inp["token_type_ids"].astype(np.int64)
    mt = inp["match_type_ids"].astype(np.int64)
    ti = inp["type_idx"].astype(np.int64)
    cpos = inp["col_pos"].astype(np.int64)
    cidx = inp["col_idx"].astype(np.int64)
    hlen = inp["header_len"].astype(np.int64)

    pre = [
        _prep_core(core, iid, hdr, tt, mt, ti, cpos, cidx, hlen, bf16)
        for core in range(NCORES)
    ]
    # static pair-group count shared by all cores (compiled in)
    Gp = max(1, max((len(p[1]) + 127) // 128 for p in pre))

    in_maps = []
    for core, (widx16, pairs, lens, posidx, tgtrow, mh, mht) in enumerate(pre):
        hflat = np.full(2 * Gp * 128, ZROW, np.int64)
        M = np.zeros((128, Gp * 128), dtype=np.float32)
        for q, (s, t0, t1) in enumerate(pairs):
            p, gp = q % 128, q // 128
            hflat[2 * gp * 128 + p] = t0
            hflat[(2 * gp + 1) * 128 + p] = t1
            M[p, gp * 128 + s] = 1.0 / max(int(lens[s]), 1)
        hidx = _wrap16(hflat)
        idx16 = np.ascontiguousarray(
            np.concatenate([widx16, hidx], axis=1)
        )
        idx32 = np.ascontiguousarray(
            np.stack([posidx, tgtrow], axis=1).astype(np.int32)
        )

        cA = np.ascontiguousarray(
            np.concatenate([eye, posmat, M.astype(bf16)], axis=1)
        )
        cB = np.ascontiguousarray(
            np.concatenate([mh, small16, mht], axis=1)
        )
        m = dict(
            word_aug=word_aug, pos_aug=pos16, idx16=idx16, idx32=idx32,
            cA=cA, cB=cB,
        )
        if not skip_affine:
            m["lnw"] = lnw
            m["lnb"] = lnb
        in_maps.append(m)
    return in_maps, skip_affine, Gp


def get_nc(skip_affine, Gp):
    key = (skip_affine, Gp)
    if key not in _NC_CACHE:
        _NC_CACHE[key] = _build_nc(skip_affine, Gp)
    return _NC_CACHE[key]


def run_hw(inputs, trace=False, trace_cores=None):
    """Returns (out [B,S,H] f32, BassKernelResults)."""
    from concourse.bass_utils import run_bass_kernel_spmd

    in_maps, skip_affine, Gp = make_in_maps(inputs)
    nc = get_nc(skip_affine, Gp)
    res = run_bass_kernel_spmd(
        nc, in_maps, core_ids=list(range(NCORES)), trace=trace,
        trace_cores=trace_cores,
    )
    out = np.concatenate([res.results[c]["out"] for c in range(NCORES)], axis=0)
    return out, res


def kernel(**inputs) -> np.ndarray:
    out, _ = run_hw(inputs, trace=False)
    return out
